# revision 43
# baseline (speedup 1.0000x reference)
"""BTSPAttention Trainium2 kernel for 8 NeuronCores (self-contained).

Usage: kernel(**inputs) -> np.ndarray  (full [2,2048,1024] float32 output)

Sharding: 8 cores = 2 batches x 4 head-groups (4 heads each).

v4 architecture -- single fused emission stream, ACT(exp)-limited:
  All matmul operands are bf16 (fp32r streams 2 PE passes on HW; bf16 is 1
  pass + FWL weight loads).  The scalar engine is the critical resource
  (~128 exps of [128,1024] at ~1.1us each), so everything else is arranged
  to keep its FIFO fed with exps back-to-back:
  - K-projection runs first (kT needed by every score chunk), then Q/V for
    slice 0, then attention begins; the remaining Q/V slices re-DMA their
    x chunks and run as deferred closures drained between attention
    iterations (engine FIFOs execute in emission order).
  - Scores per (head-pair, qs, k-chunk c): one [128,1024] PSUM tile, two
    K=64 matmuls in disjoint PE row groups (concurrent), one FD=1024 exp
    for both heads; pure-clip chunks fold the time-bias into the exp bias,
    boundary chunks multiply by a precomputed bf16 Toeplitz block on DVE.
  - ctx accum [65,512] = [V_h|1]^T @ P (row 64 = softmax denominator);
    evacuated to SBUF bf16 immediately; normalization (exp(-ln(sum)) on
    ACT, PE ones-broadcast, DVE multiply) is deferred onto the queue.
  - Out-projection at K=128: ctxn2 [128, T+16] holds the head's context
    twice -- rows 0:64 at column q+1, rows 64:128 (DMA-duplicated) at
    column q -- so the AP [128 part, col 2u+1 step 16] is exactly the
    (tf=2u | tf=2u+1, j) contraction block; 8 K=128 matmuls per (head, ds)
    replace 16 K=64 ones.  wog3[64*(tf%2)+j, tf//2, do] = Wo.T grouping.
  - Zero-bias fast path (host-detected): Q/K evacuations are DVE copies;
    the general path uses ACT Identity+bias.  bk and is_gate are dropped
    exactly (softmax shift invariance); bv and bo applied on the host.
PSUM tags: "sc" 2x[128,1024] (scores), "ctx" 2x[65,512] (ctx accum),
"bc" + "y" 1x[128,512]-class each (K/Q/V deferred accums, norm
broadcast, out-proj accum).  8 banks exactly.  NOTE: matmul start=True
clears PSUM at BANK granularity -- never pack two accumulation groups
into one 2KB bank (the V-pass runs as two tb-pair sub-passes for this).
"""

import numpy as np
import ml_dtypes

import sys as _sys
if '/opt/trn_rl_repo' not in _sys.path:
    _sys.path.insert(0, '/opt/trn_rl_repo')


import concourse.bass as bass
import concourse.tile as tile
from concourse import bacc
from concourse import mybir

F32 = mybir.dt.float32
F32R = mybir.dt.float32r
BF16 = mybir.dt.bfloat16
AF = mybir.ActivationFunctionType

T = 2048
D = 1024
HD = 64
TB_LEN = 500
NKC = 16   # k chunks of 128
NDC = 8    # D chunks of 128

# ---- structural chunk classification (depends only on the clip pattern) ----
# scoresT chunk (c, qs): k in [128c, 128c+128), q in [512qs, 512qs+512).
# E[k, q] = exp(sig * tb[clip(k - q + 250, 0, 499)]).
# pure-low  (idx pinned 0):   k - q <= -250 everywhere  -> E = exp(sig*tb[0])
# pure-high (idx pinned 499): k - q >= 249 everywhere   -> E = exp(sig*tb[499])
def _classify(c, qs):
    kmin, kmax = 128 * c, 128 * c + 127
    qmin, qmax = 512 * qs, 512 * qs + 511
    if kmax - qmin <= -250:
        return 'low'
    if kmin - qmax >= 249:
        return 'high'
    return 'boundary'

CHUNK_CLS = {(c, qs): _classify(c, qs) for c in range(NKC) for qs in range(4)}
BOUNDARY = [(c, qs) for qs in range(4) for c in range(NKC)
            if CHUNK_CLS[(c, qs)] == 'boundary']
BIDX = {cq: i for i, cq in enumerate(BOUNDARY)}
NB = len(BOUNDARY)  # 28
OUTPROJ_K64 = False  # debug bisect: revert to v3-style K=64 out-projection
DEBUG_DUMP = False   # dump head-0 ctxn2 layout to 'dbg' output


def host_prep(inputs):
    """Returns (in_maps for 8 cores, postprocess-closure, biases_zero)."""
    x = np.asarray(inputs["x"], np.float32)
    Wq = np.asarray(inputs["Wq"], np.float32)
    Wk = np.asarray(inputs["Wk"], np.float32)
    Wv = np.asarray(inputs["Wv"], np.float32)
    Wo = np.asarray(inputs["Wo"], np.float32)
    bq = np.asarray(inputs["bq"], np.float32)
    bk = np.asarray(inputs["bk"], np.float32)
    bv = np.asarray(inputs["bv"], np.float32)
    bo = np.asarray(inputs["bo"], np.float32)
    et = float(np.asarray(inputs["et_gate"], np.float32).reshape(()))
    tb = np.asarray(inputs["time_bias"], np.float32).reshape(-1)
    assert tb.shape == (TB_LEN,)
    # bk shifts every score of a query by the same amount -> softmax
    # invariant -> dropped exactly.  bq is not invariant; when nonzero the
    # program variant with ACT-bias evacuation is used.
    biases_zero = not (np.any(bq) or np.any(bk))

    sig = 1.0 / (1.0 + np.exp(-et))
    idx = np.clip(np.arange(T)[:, None] - np.arange(T)[None, :] + TB_LEN // 2,
                  0, TB_LEN - 1)              # [k, q]
    E = np.exp(np.float32(sig) * tb[idx]).astype(np.float32)
    # boundary-chunk Toeplitz table, duplicated halves (head A | head B
    # of a pair share the same (c, qs) block): [128, NB, 1024]
    ebb = np.empty((128, NB, 1024), np.float32)
    for i, (c, qs) in enumerate(BOUNDARY):
        blk = E[128 * c:128 * c + 128, 512 * qs:512 * qs + 512]
        ebb[:, i, 0:512] = blk
        ebb[:, i, 512:1024] = blk
    ebb = ebb.astype(ml_dtypes.bfloat16)

    # exp bias for pure chunks: log E = sig * tb[0 or 499]
    pb = np.zeros((128, 2), np.float32)
    pb[:, 0] = sig * tb[0]           # pure-low
    pb[:, 1] = sig * tb[TB_LEN - 1]  # pure-high

    # wog3[64*(tf%2)+j, tf//2, do] = Wo.T[64tf+j, do]: the (tf-parity, j)
    # contraction grouping for the K=128 out-projection
    wg = np.ascontiguousarray(Wo.T.reshape(16, 64, D).transpose(1, 0, 2))  # [j, tf, do]
    wog3 = np.zeros((128, 8, D), np.float32)
    for tf in range(16):
        wog3[64 * (tf % 2):64 * (tf % 2) + 64, tf // 2, :] = wg[:, tf, :]
    wog3 = wog3.astype(ml_dtypes.bfloat16)

    def chunk_w(Wl):  # Wl [256, 1024] -> [128, 8, 256]: [p, c, m] = Wl[m, c*128+p]
        return np.ascontiguousarray(
            Wl.T.reshape(NDC, 128, 256).transpose(1, 0, 2)
        ).astype(ml_dtypes.bfloat16)

    in_maps = []
    for core in range(8):
        b, hg = core // 4, core % 4
        sl = slice(hg * 256, (hg + 1) * 256)
        bqk = np.stack([bq[sl][:128], bq[sl][128:],
                        bk[sl][:128], bk[sl][128:]], axis=1)  # [128, 4]
        in_maps.append({
            "xT": np.ascontiguousarray(x[b].T).astype(ml_dtypes.bfloat16),
            "wq": chunk_w(Wq[sl]),
            "wk": chunk_w(Wk[sl]),
            "wv": chunk_w(Wv[sl]),
            "wog": wog3,
            "bqk": np.ascontiguousarray(bqk, np.float32),
            "pb": pb,
            "ones": np.ones((128, 64), np.float32),
            "eb": ebb,
        })

    corr = np.einsum("hj,jfd->hd", bv.reshape(16, HD), wg).astype(np.float32)  # per global head

    def post(results):
        out = np.empty((2, T, D), np.float32)
        for core in range(8):
            b, hg = core // 4, core % 4
            yc = results[core]["y"]  # [512, 1024]
            for hl in range(4):
                h = hg * 4 + hl
                rows = (h % 8) * 256 + b * 128
                out[h // 8, rows:rows + 128, :] = (
                    yc[hl * 128:(hl + 1) * 128] + corr[h][None, :] + bo[None, :]
                )
        return out

    return in_maps, post, biases_zero


def expected_core(inputs, core):
    """Numpy model of one core's device output (for sim checks)."""
    m, _, _ = host_prep(inputs)
    im = m[core]
    et = float(np.asarray(inputs["et_gate"], np.float32).reshape(()))
    tb = np.asarray(inputs["time_bias"], np.float32).reshape(-1)
    sig = 1.0 / (1.0 + np.exp(-et))
    idx = np.clip(np.arange(T)[:, None] - np.arange(T)[None, :] + TB_LEN // 2,
                  0, TB_LEN - 1)
    E = np.exp(np.float32(sig) * tb[idx]).astype(np.float32)
    y = np.zeros((512, 1024), np.float32)
    bqk = im["bqk"]
    xT = np.asarray(im["xT"], np.float32)

    def _bf(a):
        return a.astype(ml_dtypes.bfloat16).astype(np.float32)

    wq = np.asarray(im["wq"], np.float32)
    wk = np.asarray(im["wk"], np.float32)
    wv = np.asarray(im["wv"], np.float32)
    Wq_l = np.concatenate([wq[:, c, :] for c in range(NDC)], axis=0)  # [1024, 256] = Wl.T
    Wk_l = np.concatenate([wk[:, c, :] for c in range(NDC)], axis=0)
    Wv_l = np.concatenate([wv[:, c, :] for c in range(NDC)], axis=0)
    QT = _bf(Wq_l.T @ xT + np.concatenate([bqk[:, 0], bqk[:, 1]])[:, None])
    KT = _bf(Wk_l.T @ xT + np.concatenate([bqk[:, 2], bqk[:, 3]])[:, None])
    V = xT.T @ Wv_l
    wog3 = np.asarray(im["wog"], np.float32)  # [128, 8, 1024]
    wg = np.zeros((64, 16, D), np.float32)
    for tf in range(16):
        wg[:, tf, :] = wog3[64 * (tf % 2):64 * (tf % 2) + 64, tf // 2, :]
    for hl in range(4):
        qh_ = QT[hl * 64:(hl + 1) * 64]
        kh = KT[hl * 64:(hl + 1) * 64]
        P = np.exp(0.125 * (kh.T @ qh_)) * E
        c = (V[:, hl * 64:(hl + 1) * 64].T @ P) / P.sum(axis=0)[None, :]  # [64, q]
        cn = _bf(c)
        g = cn.reshape(64, 128, 16)
        y[hl * 128:(hl + 1) * 128] = np.einsum("jcf,jfd->cd", g, wg)
    return y


def build_program(repeats=1, biases_zero=True):
    nc = bacc.Bacc("TRN2", target_bir_lowering=False, debug=False,
                   dynamic_dma_scratch_size=4096)

    # All activation functions used here (Exp, Ln, Copy, Identity) live in
    # the natural_log_exp_and_others table set, but walrus's first-match set
    # selection would ping-pong between exp_and_others and natural_log
    # (one ~1.3us ACT_TABLE_LOAD per Ln/Exp alternation, 17 loads/kernel).
    # Restrict the offered tables so a single load covers the whole kernel.
    import types as _types

    def _single_act_set(self):
        has_activation = any(
            isinstance(i, mybir.InstActivation)
            for b in self.main_func.blocks
            for i in b.instructions
        )
        if not has_activation:
            return
        from concourse.hw_specs import get_activation_tables
        tables = [(n, f if n == 'natural_log_exp_and_others' else set())
                  for n, f in get_activation_tables(self.m.arch).items()]
        assert any(f for _, f in tables), "natural_log_exp_and_others missing"
        bacc._bass_rust.insert_act_table_loads(self, tables)

    nc.insert_act_table_loads = _types.MethodType(_single_act_set, nc)
    xT = nc.dram_tensor("xT", [D, T], BF16, kind="ExternalInput").ap()
    wq_d = nc.dram_tensor("wq", [128, NDC, 256], BF16, kind="ExternalInput").ap()
    wk_d = nc.dram_tensor("wk", [128, NDC, 256], BF16, kind="ExternalInput").ap()
    wv_d = nc.dram_tensor("wv", [128, NDC, 256], BF16, kind="ExternalInput").ap()
    wog_d = nc.dram_tensor("wog", [128, 8, D], BF16, kind="ExternalInput").ap()
    bqk_d = nc.dram_tensor("bqk", [128, 4], F32, kind="ExternalInput").ap()
    pb_d = nc.dram_tensor("pb", [128, 2], F32, kind="ExternalInput").ap()
    ones_d = nc.dram_tensor("ones", [128, 64], F32R, kind="ExternalInput").ap()
    eb_d = nc.dram_tensor("eb", [128, NB, 1024], BF16, kind="ExternalInput").ap()
    y_d = nc.dram_tensor("y", [512, D], F32, kind="ExternalOutput").ap()
    if DEBUG_DUMP:
        dbg_d = nc.dram_tensor("dbg", [128, T + 16], BF16,
                               kind="ExternalOutput").ap()
        dbgv_d = nc.dram_tensor("dbgv", [128, NKC, 4, 65], BF16,
                                kind="ExternalOutput").ap()
        dbgq_d = nc.dram_tensor("dbgq", [128, 2, T], BF16,
                                kind="ExternalOutput").ap()
        dbgk_d = nc.dram_tensor("dbgk", [128, 2, T], BF16,
                                kind="ExternalOutput").ap()

    import collections as _collections

    with tile.TileContext(nc) as tc:
        with (
            tc.tile_pool(name="const", bufs=1) as const,
            tc.tile_pool(name="persist", bufs=1) as persist,
            tc.tile_pool(name="xp", bufs=8) as xp,
            tc.tile_pool(name="pp", bufs=8) as pp,
            tc.tile_pool(name="ctxnp", bufs=4) as ctxnp,
            tc.tile_pool(name="ctxsp", bufs=4) as ctxsp,
            tc.tile_pool(name="rbp", bufs=2) as rbp,
            tc.tile_pool(name="bcp", bufs=2) as bcp,
            tc.tile_pool(name="yevac", bufs=4) as yevac,
            tc.tile_pool(name="scps", bufs=2, space="PSUM") as scps,
            tc.tile_pool(name="ctxps", bufs=2, space="PSUM") as ctxps,
        ):
            # ---- constants ----
            wq = const.tile([128, NDC, 256], BF16, tag="wq")
            wk = const.tile([128, NDC, 256], BF16, tag="wk")
            wv = const.tile([128, NDC, 256], BF16, tag="wv")
            wog = const.tile([128, 8, D], BF16, tag="wog")
            bqk = const.tile([128, 4], F32, tag="bqk")
            pbt = const.tile([128, 2], F32, tag="pb")
            ones_r = const.tile([128, 64], F32R, tag="ones_r")
            eb = const.tile([128, NB, 1024], BF16, tag="eb")
            # DMA order = need order: wk (K-pass first), then wq/wv, then
            # the attention-phase constants (first-qs boundary eb blocks
            # early, the rest + wog behind everything x-critical).
            nc.sync.dma_start(wk[:], wk_d[:])
            nc.sync.dma_start(bqk[:], bqk_d[:])
            nc.sync.dma_start(ones_r[:], ones_d[:])

            def mid_const_dmas():
                nc.sync.dma_start(wq[:], wq_d[:])
                nc.sync.dma_start(wv[:], wv_d[:])
                nc.sync.dma_start(pbt[:], pb_d[:])
                # first 6 eb entries are the qs=0 boundary blocks
                nc.sync.dma_start(eb[:, 0:6, :], eb_d[:, 0:6, :])

            def late_const_closures():
                # eb thirds + wog as closures so the deferred-slice x
                # re-DMAs interleave with them in the DMA queue instead of
                # stalling ~20us behind 7.8MB of attention constants
                cls = []
                nq = (NB - 6 + 2) // 3
                for i0 in range(6, NB, nq):
                    i1 = min(i0 + nq, NB)
                    cls.append(lambda i0=i0, i1=i1: nc.sync.dma_start(
                        eb[:, i0:i1, :], eb_d[:, i0:i1, :]))
                cls.append(lambda: nc.sync.dma_start(wog[:], wog_d[:]))
                return cls

            for _r in range(repeats):
                qT = [persist.tile([128, T], BF16, tag=f"qT{i}", name=f"qT{i}_{_r}") for i in range(2)]
                kT = [persist.tile([128, T], BF16, tag=f"kT{i}", name=f"kT{i}_{_r}") for i in range(2)]
                v_sb = persist.tile([128, NKC, 4, 65], BF16, tag="v_sb")
                nc.vector.memset(v_sb[:], 1.0)

                def xc_dmas(s):
                    xcs = []
                    for c in range(NDC):
                        xc = xp.tile([128, 512], BF16, tag="xc",
                                     name=f"xc_{_r}_{s}_{c}")
                        nc.sync.dma_start(
                            xc[:], xT[c * 128:(c + 1) * 128,
                                      s * 512:(s + 1) * 512])
                        xcs.append(xc)
                    return xcs

                ssl = lambda s: slice(s * 512, (s + 1) * 512)

                def k_alloc(s):
                    return [ctxps.tile([128, 512], F32, tag=t, bufs=1,
                                       name=f"kps{hp}_{_r}_{s}")
                            for hp, t in ((0, "bc"), (1, "y"))]

                def k_mms(s, xcs, k_ps, c0, c1):
                    for c in range(c0, c1):
                        st, sp = (c == 0), (c == NDC - 1)
                        for hp in range(2):
                            nc.tensor.matmul(
                                k_ps[hp][:],
                                wk[:, c, hp * 128:(hp + 1) * 128],
                                xcs[c][:], start=st, stop=sp)

                def k_evac(s, k_ps):
                    for hp in range(2):
                        if biases_zero:
                            nc.vector.tensor_copy(
                                kT[hp][:, ssl(s)], k_ps[hp][:])
                        else:
                            nc.scalar.activation(
                                kT[hp][:, ssl(s)], k_ps[hp][:],
                                AF.Identity, bias=bqk[:, 2 + hp:3 + hp])

                def k_slice(s, xcs):
                    k_ps = k_alloc(s)
                    k_mms(s, xcs, k_ps, 0, NDC)
                    k_evac(s, k_ps)

                def q_mms(s, xcs, q_ps, c0, c1):
                    for c in range(c0, c1):
                        st, sp = (c == 0), (c == NDC - 1)
                        for hp in range(2):
                            nc.tensor.matmul(
                                q_ps[hp][:],
                                wq[:, c, hp * 128:(hp + 1) * 128],
                                xcs[c][:], start=st, stop=sp)

                def q_evac(s, q_ps):
                    for hp in range(2):
                        if biases_zero:
                            nc.vector.tensor_copy(
                                qT[hp][:, ssl(s)], q_ps[hp][:])
                        else:
                            nc.scalar.activation(
                                qT[hp][:, ssl(s)], q_ps[hp][:],
                                AF.Identity, bias=bqk[:, hp:hp + 1])

                # v accumulators must be one-accumulation-group-per-PSUM-bank:
                # start=True clears at bank granularity, so packing two tb
                # groups into one 2KB bank clobbers the partner's partials
                def v_mms(s, xcs, v_ps, tbp, c0, c1):
                    for c in range(c0, c1):
                        st, sp = (c == 0), (c == NDC - 1)
                        for i in range(2):
                            tb = 2 * tbp + i
                            nc.tensor.matmul(
                                v_ps[i][:, 0:256],
                                xcs[c][:, tb * 128:(tb + 1) * 128],
                                wv[:, c, :], start=st, stop=sp)

                def v_copy(s, v_ps, tbp):
                    for i in range(2):
                        tb = 2 * tbp + i
                        kc = s * 4 + tb
                        vsrc = v_ps[i][:, 0:256].rearrange(
                            "p (h j) -> p h j", h=4)
                        nc.vector.tensor_copy(v_sb[:, kc, :, 0:64], vsrc[:])

                def v_closures(s, box):
                    cls = []
                    for tbp in range(2):
                        def valloc(s=s, box=box, tbp=tbp):
                            box['v'] = [ctxps.tile([128, 256], F32, tag=t,
                                                   bufs=1,
                                                   name=f"vps{i}_{_r}_{s}_{tbp}")
                                        for i, t in ((0, "bc"), (1, "y"))]
                        cls.append(valloc)
                        for c0 in range(0, NDC, 4):
                            cls.append(lambda s=s, box=box, c0=c0, tbp=tbp:
                                       v_mms(s, box['x'], box['v'], tbp,
                                             c0, c0 + 4))
                        cls.append(lambda s=s, box=box, tbp=tbp:
                                   v_copy(s, box['v'], tbp))
                    return cls

                def q_closures(s, box):
                    def qalloc(s=s, box=box):
                        box['q'] = [ctxps.tile([128, 512], F32, tag=t,
                                               bufs=1, name=f"qps{hp}_{_r}_{s}")
                                    for hp, t in ((0, "bc"), (1, "y"))]

                    cls = [qalloc]
                    for c0 in range(0, NDC, 4):
                        cls.append(lambda s=s, box=box, c0=c0:
                                   q_mms(s, box['x'], box['q'], c0, c0 + 4))
                    cls.append(lambda s=s, box=box: q_evac(s, box['q']))
                    return cls

                def kv_slice_closures(s, with_q):
                    """Deferred K+V (+Q for slice 1) for a slice: one x
                    re-DMA shared by all passes, drained during qs0."""
                    box = {}

                    def dmas(s=s, box=box):
                        box['x'] = xc_dmas(s)
                        box['k'] = k_alloc(s)

                    cls = [dmas]
                    for c0 in range(0, NDC, 4):
                        cls.append(lambda s=s, box=box, c0=c0:
                                   k_mms(s, box['x'], box['k'], c0, c0 + 4))
                    cls.append(lambda s=s, box=box: k_evac(s, box['k']))
                    cls += v_closures(s, box)
                    if with_q:
                        cls += q_closures(s, box)
                    return cls

                def q_slice_closures(s):
                    """Deferred Q-only for a slice (own x re-DMA), drained
                    one q-slice before it's consumed."""
                    box = {}
                    cls = [lambda s=s, box=box: box.__setitem__(
                        'x', xc_dmas(s))]
                    cls += q_closures(s, box)
                    return cls

                # ---- prelude: K0 + Q0 only -- everything else deferred
                xcs0 = xc_dmas(0)
                mid_const_dmas()
                k_slice(0, xcs0)
                q_ps0 = [ctxps.tile([128, 512], F32, tag=t, bufs=1,
                                    name=f"qps{hp}_{_r}_p0")
                         for hp, t in ((0, "bc"), (1, "y"))]
                q_mms(0, xcs0, q_ps0, 0, NDC)
                q_evac(0, q_ps0)
                ebcls = late_const_closures()

                # ---- fused attention + deferred QKV/norm/out-proj ----
                pending = _collections.deque()

                def drain(n):
                    k = min(n, len(pending))
                    for _ in range(k):
                        pending.popleft()()
                    return k

                dbg_keep = []
                for hp in range(2):
                    hlA, hlB = 2 * hp, 2 * hp + 1
                    # ctxn2[0:64, 1+q] and (duplicated) [64:128, q] per head
                    ctxn2 = [ctxnp.tile([128, T + 16], BF16, tag="ctxn",
                                        name=f"ctxn2_{_r}_{hp}_{i}")
                             for i in range(2)]
                    if hp == 0 and not dbg_keep:
                        dbg_keep.append(ctxn2[0])

                    for qs in range(4):
                        if hp == 0 and qs == 0:
                            # every qs scans all 16 k-chunks, so K and V for
                            # ALL remaining slices must land during qs0
                            # (scores chunk c needs kT[c], AV needs v_sb[c]);
                            # Q1 rides along on slice 1's x re-DMA, Q2/Q3 are
                            # deferred to later q-slices to keep qs0's PE
                            # load down; eb-table DMA thirds interleave so x
                            # transfers aren't queued behind them
                            box0 = {'x': xcs0}
                            pending.extend(v_closures(0, box0))
                            for s in range(1, 4):
                                pending.extend(kv_slice_closures(s, s == 1))
                                pending.append(ebcls[s - 1])
                            pending.append(ebcls[3])
                        if hp == 0 and qs in (1, 2):
                            pending.extend(q_slice_closures(qs + 1))
                        ctxA = ctxps.tile([65, 512], F32, tag="ctx",
                                          name=f"ctxA_{_r}_{hp}_{qs}")
                        ctxB = ctxps.tile([65, 512], F32, tag="ctx",
                                          name=f"ctxB_{_r}_{hp}_{qs}")
                        pts = {}

                        def emit_av(cc, hlA=hlA, hlB=hlB, ctxA=ctxA,
                                    ctxB=ctxB, pts=pts):
                            pm = pts.pop(cc)
                            st, sp = (cc == 0), (cc == NKC - 1)
                            nc.tensor.matmul(
                                ctxA[:], v_sb[:, cc, hlA, :],
                                pm[:, 0:512], start=st, stop=sp)
                            nc.tensor.matmul(
                                ctxB[:], v_sb[:, cc, hlB, :],
                                pm[:, 512:1024], start=st, stop=sp)

                        qsl = slice(qs * 512, (qs + 1) * 512)
                        warm_pad = False
                        for c in range(NKC):
                            sc = scps.tile([128, 1024], F32, tag="sc")
                            if warm_pad:
                                # ACT-bound iterations leave the PE at ~55%
                                # duty; the HAM clock gate then re-throttles
                                # it to 1.2GHz and the slowed matmuls become
                                # the limiter.  A throwaway [128,256] matmul
                                # into the fresh score tile (overwritten by
                                # the real scores below) keeps the duty above
                                # the gate threshold.  ~100ns@2.4GHz, fits in
                                # the ACT slack.
                                nc.tensor.matmul(
                                    sc[:, 0:256],
                                    kT[hp][:, 0:128],
                                    qT[hp][:, 0:256],
                                    start=True, stop=True)
                            nc.tensor.matmul(
                                sc[:, 0:512],
                                kT[hp][0:64, c * 128:(c + 1) * 128],
                                qT[hp][0:64, qsl],
                                start=True, stop=True)
                            nc.tensor.matmul(
                                sc[:, 512:1024],
                                kT[hp][64:128, c * 128:(c + 1) * 128],
                                qT[hp][64:128, qsl],
                                start=True, stop=True)
                            p_t = pp.tile([128, 1024], BF16, tag="p")
                            cls = CHUNK_CLS[(c, qs)]
                            if cls == 'boundary':
                                nc.scalar.activation(p_t[:], sc[:], AF.Exp,
                                                     scale=0.125)
                                nc.vector.tensor_mul(
                                    p_t[:], p_t[:], eb[:, BIDX[(c, qs)], :])
                            else:
                                col = 0 if cls == 'low' else 1
                                nc.scalar.activation(p_t[:], sc[:], AF.Exp,
                                                     scale=0.125,
                                                     bias=pbt[:, col:col + 1])
                            pts[c] = p_t
                            if c >= 1:
                                emit_av(c - 1)
                            # qs0 must absorb the K/V bundles fast (hard
                            # dependencies at c=4s); elsewhere 1/iter keeps
                            # the PE from starving ACT in bursts.  When the
                            # queue runs dry the NEXT iteration emits a
                            # warm-keeper matmul instead.
                            if hp == 0 and qs == 0:
                                done = drain(5 if c < 12 else 2)
                            else:
                                done = drain(1)
                            warm_pad = (done == 0)
                        emit_av(NKC - 1)

                        # evacuate ctx (+denominator rows) to one SBUF bf16
                        # tile [65, 1024] (A | B) right away; norm deferred
                        cs = ctxsp.tile([65, 1024], BF16, tag="ctxs",
                                        name=f"ctxs_{_r}_{hp}_{qs}")
                        nc.vector.tensor_copy(cs[:, 0:512], ctxA[:])
                        nc.vector.tensor_copy(cs[:, 512:1024], ctxB[:])

                        # normalization: 1/sum = exp(-ln(sum)) on ACT,
                        # batched over both heads (one FD=1024 Ln + Exp)
                        def norm_ops(cs=cs, ctxn2=ctxn2, qs=qs, qsl=qsl,
                                     key=f"{_r}_{hp}_{qs}"):
                            rbl = rbp.tile([65, 1024], F32, tag="rbl",
                                           name=f"rbl_{key}")
                            rbr = rbp.tile([65, 1024], F32R, tag="rbr",
                                           name=f"rbr_{key}")
                            nc.scalar.activation(rbl[64:65, 0:1024],
                                                 cs[64:65, :], AF.Ln)
                            nc.scalar.activation(rbr[64:65, 0:1024],
                                                 rbl[64:65, 0:1024],
                                                 AF.Exp, scale=-1.0)
                            for half in range(2):
                                hsl = slice(half * 512, (half + 1) * 512)
                                bc_ps = ctxps.tile([64, 512], F32, tag="bc",
                                                   bufs=1,
                                                   name=f"bcps_{key}_{half}")
                                nc.tensor.matmul(
                                    bc_ps[:],
                                    ones_r[64:65, 0:64],
                                    rbr[64:65, hsl],
                                    start=True, stop=True)
                                bc_sb = bcp.tile([64, 512], BF16, tag="bc",
                                                 name=f"bcsb_{key}_{half}")
                                nc.vector.tensor_copy(bc_sb[:, 0:512],
                                                      bc_ps[:])
                                nc.vector.tensor_mul(
                                    ctxn2[half][0:64,
                                                1 + qs * 512:1 + qs * 512 + 512],
                                    cs[0:64, hsl], bc_sb[:, 0:512])
                                # duplicate (unshifted) into rows 64:128
                                nc.sync.dma_start(
                                    ctxn2[half][64:128, qsl],
                                    ctxn2[half][0:64,
                                                1 + qs * 512:
                                                1 + qs * 512 + 512])
                        pending.append(norm_ops)

                    # K=128 out-projections: 8 u-chunks x 2 ds per head
                    for hoff in range(2):
                        hl = 2 * hp + hoff
                        r2 = ctxn2[hoff].rearrange("p (tc s) -> p s tc", s=16)
                        for ds in range(2):
                            ypsb = []

                            def yalloc(hl=hl, ds=ds, ypsb=ypsb):
                                ypsb.append(ctxps.tile(
                                    [128, 512], F32, tag="y", bufs=1,
                                    name=f"yps_{_r}_{hl}_{ds}"))

                            if OUTPROJ_K64:
                                def ymm(u0, r2=r2, ds=ds, ypsb=ypsb):
                                    for tf in range(2 * u0, 2 * u0 + 8):
                                        par = tf % 2
                                        nc.tensor.matmul(
                                            ypsb[0][:],
                                            r2[64 * par:64 * par + 64,
                                               tf + 1 - par, 0:128],
                                            wog[64 * par:64 * par + 64,
                                                tf // 2,
                                                ds * 512:(ds + 1) * 512],
                                            start=(tf == 0), stop=(tf == 15))
                            else:
                                def ymm(u0, r2=r2, ds=ds, ypsb=ypsb):
                                    for u in range(u0, u0 + 4):
                                        nc.tensor.matmul(
                                            ypsb[0][:],
                                            r2[:, 2 * u + 1, 0:128],
                                            wog[:, u, ds * 512:(ds + 1) * 512],
                                            start=(u == 0), stop=(u == 7))

                            def yout(hl=hl, ds=ds, ypsb=ypsb):
                                ysb = yevac.tile([128, 512], F32, tag="y",
                                                 name=f"ysb_{_r}_{hl}_{ds}")
                                nc.vector.tensor_copy(ysb[:], ypsb[0][:])
                                nc.sync.dma_start(
                                    y_d[hl * 128:(hl + 1) * 128,
                                        ds * 512:(ds + 1) * 512],
                                    ysb[:])

                            pending.append(yalloc)
                            for u0 in (0, 4):
                                pending.append(
                                    lambda u0=u0, ymm=ymm: ymm(u0))
                            pending.append(yout)

                # tail: whatever the last head pair's attention didn't absorb
                drain(len(pending))
                if DEBUG_DUMP:
                    nc.sync.dma_start(dbg_d[:], dbg_keep[0][:])
                    nc.sync.dma_start(dbgv_d[:], v_sb[:])
                    for i in range(2):
                        nc.sync.dma_start(dbgq_d[:, i, :], qT[i][:])
                        nc.sync.dma_start(dbgk_d[:, i, :], kT[i][:])
    nc.compile()
    return nc


_PROGRAM_CACHE = {}


def _get_program(repeats=1, biases_zero=True):
    key = (repeats, biases_zero)
    if key not in _PROGRAM_CACHE:
        _PROGRAM_CACHE[key] = build_program(repeats=repeats,
                                            biases_zero=biases_zero)
    return _PROGRAM_CACHE[key]


def kernel(**inputs):
    from concourse.bass_utils import run_bass_kernel_spmd
    in_maps, post, biases_zero = host_prep(inputs)
    nc = _get_program(repeats=1, biases_zero=biases_zero)
    res = run_bass_kernel_spmd(nc, in_maps, list(range(8)))
    return post(res.results)


# revision 48
# speedup vs baseline: 1.0239x; 1.0239x over previous
"""BTSPAttention Trainium2 kernel for 8 NeuronCores (self-contained).

Usage: kernel(**inputs) -> np.ndarray  (full [2,2048,1024] float32 output)

Sharding: 8 cores = 2 batches x 4 head-groups (4 heads each).

v4 architecture -- single fused emission stream, ACT(exp)-limited:
  All matmul operands are bf16 (fp32r streams 2 PE passes on HW; bf16 is 1
  pass + FWL weight loads).  The scalar engine is the critical resource
  (~128 exps of [128,1024] at ~1.1us each), so everything else is arranged
  to keep its FIFO fed with exps back-to-back:
  - K-projection runs first (kT needed by every score chunk), then Q/V for
    slice 0, then attention begins; the remaining Q/V slices re-DMA their
    x chunks and run as deferred closures drained between attention
    iterations (engine FIFOs execute in emission order).
  - Scores per (head-pair, qs, k-chunk c): one [128,1024] PSUM tile, two
    K=64 matmuls in disjoint PE row groups (concurrent), one FD=1024 exp
    for both heads; pure-clip chunks fold the time-bias into the exp bias,
    boundary chunks multiply by a precomputed bf16 Toeplitz block on DVE.
  - ctx accum [65,512] = [V_h|1]^T @ P (row 64 = softmax denominator);
    evacuated to SBUF bf16 immediately; normalization (exp(-ln(sum)) on
    ACT, PE ones-broadcast, DVE multiply) is deferred onto the queue.
  - Out-projection at K=128: ctxn2 [128, T+16] holds the head's context
    twice -- rows 0:64 at column q+1, rows 64:128 (DMA-duplicated) at
    column q -- so the AP [128 part, col 2u+1 step 16] is exactly the
    (tf=2u | tf=2u+1, j) contraction block; 8 K=128 matmuls per (head, ds)
    replace 16 K=64 ones.  wog3[64*(tf%2)+j, tf//2, do] = Wo.T grouping.
  - Zero-bias fast path (host-detected): Q/K evacuations are DVE copies;
    the general path uses ACT Identity+bias.  bk and is_gate are dropped
    exactly (softmax shift invariance); bv and bo applied on the host.
PSUM tags: "sc" 2x[128,1024] (scores), "ctx" 2x[65,512] (ctx accum),
"bc" + "y" 1x[128,512]-class each (K/Q/V deferred accums, norm
broadcast, out-proj accum).  8 banks exactly.  NOTE: matmul start=True
clears PSUM at BANK granularity -- never pack two accumulation groups
into one 2KB bank (the V-pass runs as two tb-pair sub-passes for this).
"""

import numpy as np
import ml_dtypes

import sys as _sys
if '/opt/trn_rl_repo' not in _sys.path:
    _sys.path.insert(0, '/opt/trn_rl_repo')


import concourse.bass as bass
import concourse.tile as tile
from concourse import bacc
from concourse import mybir

F32 = mybir.dt.float32
F32R = mybir.dt.float32r
BF16 = mybir.dt.bfloat16
AF = mybir.ActivationFunctionType

T = 2048
D = 1024
HD = 64
TB_LEN = 500
NKC = 16   # k chunks of 128
NDC = 8    # D chunks of 128

# ---- structural chunk classification (depends only on the clip pattern) ----
# scoresT chunk (c, qs): k in [128c, 128c+128), q in [512qs, 512qs+512).
# E[k, q] = exp(sig * tb[clip(k - q + 250, 0, 499)]).
# pure-low  (idx pinned 0):   k - q <= -250 everywhere  -> E = exp(sig*tb[0])
# pure-high (idx pinned 499): k - q >= 249 everywhere   -> E = exp(sig*tb[499])
def _classify(c, qs):
    kmin, kmax = 128 * c, 128 * c + 127
    qmin, qmax = 512 * qs, 512 * qs + 511
    if kmax - qmin <= -250:
        return 'low'
    if kmin - qmax >= 249:
        return 'high'
    return 'boundary'

CHUNK_CLS = {(c, qs): _classify(c, qs) for c in range(NKC) for qs in range(4)}
BOUNDARY = [(c, qs) for qs in range(4) for c in range(NKC)
            if CHUNK_CLS[(c, qs)] == 'boundary']
BIDX = {cq: i for i, cq in enumerate(BOUNDARY)}
NB = len(BOUNDARY)  # 28
OUTPROJ_K64 = False  # debug bisect: revert to v3-style K=64 out-projection
DEBUG_DUMP = False   # dump head-0 ctxn2 layout to 'dbg' output


def host_prep(inputs):
    """Returns (in_maps for 8 cores, postprocess-closure, biases_zero)."""
    x = np.asarray(inputs["x"], np.float32)
    Wq = np.asarray(inputs["Wq"], np.float32)
    Wk = np.asarray(inputs["Wk"], np.float32)
    Wv = np.asarray(inputs["Wv"], np.float32)
    Wo = np.asarray(inputs["Wo"], np.float32)
    bq = np.asarray(inputs["bq"], np.float32)
    bk = np.asarray(inputs["bk"], np.float32)
    bv = np.asarray(inputs["bv"], np.float32)
    bo = np.asarray(inputs["bo"], np.float32)
    et = float(np.asarray(inputs["et_gate"], np.float32).reshape(()))
    tb = np.asarray(inputs["time_bias"], np.float32).reshape(-1)
    assert tb.shape == (TB_LEN,)
    # bk shifts every score of a query by the same amount -> softmax
    # invariant -> dropped exactly.  bq is not invariant; when nonzero the
    # program variant with ACT-bias evacuation is used.
    biases_zero = not (np.any(bq) or np.any(bk))

    sig = 1.0 / (1.0 + np.exp(-et))
    idx = np.clip(np.arange(T)[:, None] - np.arange(T)[None, :] + TB_LEN // 2,
                  0, TB_LEN - 1)              # [k, q]
    E = np.exp(np.float32(sig) * tb[idx]).astype(np.float32)
    # boundary-chunk Toeplitz table, duplicated halves (head A | head B
    # of a pair share the same (c, qs) block): [128, NB, 1024]
    ebb = np.empty((128, NB, 1024), np.float32)
    for i, (c, qs) in enumerate(BOUNDARY):
        blk = E[128 * c:128 * c + 128, 512 * qs:512 * qs + 512]
        ebb[:, i, 0:512] = blk
        ebb[:, i, 512:1024] = blk
    ebb = ebb.astype(ml_dtypes.bfloat16)

    # exp bias for pure chunks: log E = sig * tb[0 or 499]
    pb = np.zeros((128, 2), np.float32)
    pb[:, 0] = sig * tb[0]           # pure-low
    pb[:, 1] = sig * tb[TB_LEN - 1]  # pure-high

    # wog3[64*(tf%2)+j, tf//2, do] = Wo.T[64tf+j, do]: the (tf-parity, j)
    # contraction grouping for the K=128 out-projection
    wg = np.ascontiguousarray(Wo.T.reshape(16, 64, D).transpose(1, 0, 2))  # [j, tf, do]
    wog3 = np.zeros((128, 8, D), np.float32)
    for tf in range(16):
        wog3[64 * (tf % 2):64 * (tf % 2) + 64, tf // 2, :] = wg[:, tf, :]
    wog3 = wog3.astype(ml_dtypes.bfloat16)

    def chunk_w(Wl):  # Wl [256, 1024] -> [128, 8, 256]: [p, c, m] = Wl[m, c*128+p]
        return np.ascontiguousarray(
            Wl.T.reshape(NDC, 128, 256).transpose(1, 0, 2)
        ).astype(ml_dtypes.bfloat16)

    in_maps = []
    for core in range(8):
        b, hg = core // 4, core % 4
        sl = slice(hg * 256, (hg + 1) * 256)
        bqk = np.stack([bq[sl][:128], bq[sl][128:],
                        bk[sl][:128], bk[sl][128:]], axis=1)  # [128, 4]
        in_maps.append({
            "xT": np.ascontiguousarray(x[b].T).astype(ml_dtypes.bfloat16),
            "wq": chunk_w(Wq[sl]),
            "wk": chunk_w(Wk[sl]),
            "wv": chunk_w(Wv[sl]),
            "wog": wog3,
            "bqk": np.ascontiguousarray(bqk, np.float32),
            "pb": pb,
            "ones": np.ones((128, 64), np.float32),
            "eb": ebb,
        })

    corr = np.einsum("hj,jfd->hd", bv.reshape(16, HD), wg).astype(np.float32)  # per global head

    def post(results):
        out = np.empty((2, T, D), np.float32)
        for core in range(8):
            b, hg = core // 4, core % 4
            yc = results[core]["y"]  # [512, 1024]
            for hl in range(4):
                h = hg * 4 + hl
                rows = (h % 8) * 256 + b * 128
                out[h // 8, rows:rows + 128, :] = (
                    yc[hl * 128:(hl + 1) * 128] + corr[h][None, :] + bo[None, :]
                )
        return out

    return in_maps, post, biases_zero


def expected_core(inputs, core):
    """Numpy model of one core's device output (for sim checks)."""
    m, _, _ = host_prep(inputs)
    im = m[core]
    et = float(np.asarray(inputs["et_gate"], np.float32).reshape(()))
    tb = np.asarray(inputs["time_bias"], np.float32).reshape(-1)
    sig = 1.0 / (1.0 + np.exp(-et))
    idx = np.clip(np.arange(T)[:, None] - np.arange(T)[None, :] + TB_LEN // 2,
                  0, TB_LEN - 1)
    E = np.exp(np.float32(sig) * tb[idx]).astype(np.float32)
    y = np.zeros((512, 1024), np.float32)
    bqk = im["bqk"]
    xT = np.asarray(im["xT"], np.float32)

    def _bf(a):
        return a.astype(ml_dtypes.bfloat16).astype(np.float32)

    wq = np.asarray(im["wq"], np.float32)
    wk = np.asarray(im["wk"], np.float32)
    wv = np.asarray(im["wv"], np.float32)
    Wq_l = np.concatenate([wq[:, c, :] for c in range(NDC)], axis=0)  # [1024, 256] = Wl.T
    Wk_l = np.concatenate([wk[:, c, :] for c in range(NDC)], axis=0)
    Wv_l = np.concatenate([wv[:, c, :] for c in range(NDC)], axis=0)
    QT = _bf(Wq_l.T @ xT + np.concatenate([bqk[:, 0], bqk[:, 1]])[:, None])
    KT = _bf(Wk_l.T @ xT + np.concatenate([bqk[:, 2], bqk[:, 3]])[:, None])
    V = xT.T @ Wv_l
    wog3 = np.asarray(im["wog"], np.float32)  # [128, 8, 1024]
    wg = np.zeros((64, 16, D), np.float32)
    for tf in range(16):
        wg[:, tf, :] = wog3[64 * (tf % 2):64 * (tf % 2) + 64, tf // 2, :]
    for hl in range(4):
        qh_ = QT[hl * 64:(hl + 1) * 64]
        kh = KT[hl * 64:(hl + 1) * 64]
        P = np.exp(0.125 * (kh.T @ qh_)) * E
        c = (V[:, hl * 64:(hl + 1) * 64].T @ P) / P.sum(axis=0)[None, :]  # [64, q]
        cn = _bf(c)
        g = cn.reshape(64, 128, 16)
        y[hl * 128:(hl + 1) * 128] = np.einsum("jcf,jfd->cd", g, wg)
    return y


def build_program(repeats=1, biases_zero=True):
    nc = bacc.Bacc("TRN2", target_bir_lowering=False, debug=False,
                   dynamic_dma_scratch_size=4096)

    # All activation functions used here (Exp, Ln, Copy, Identity) live in
    # the natural_log_exp_and_others table set, but walrus's first-match set
    # selection would ping-pong between exp_and_others and natural_log
    # (one ~1.3us ACT_TABLE_LOAD per Ln/Exp alternation, 17 loads/kernel).
    # Restrict the offered tables so a single load covers the whole kernel.
    import types as _types

    def _single_act_set(self):
        has_activation = any(
            isinstance(i, mybir.InstActivation)
            for b in self.main_func.blocks
            for i in b.instructions
        )
        if not has_activation:
            return
        from concourse.hw_specs import get_activation_tables
        tables = [(n, f if n == 'natural_log_exp_and_others' else set())
                  for n, f in get_activation_tables(self.m.arch).items()]
        assert any(f for _, f in tables), "natural_log_exp_and_others missing"
        bacc._bass_rust.insert_act_table_loads(self, tables)

    nc.insert_act_table_loads = _types.MethodType(_single_act_set, nc)
    xT = nc.dram_tensor("xT", [D, T], BF16, kind="ExternalInput").ap()
    wq_d = nc.dram_tensor("wq", [128, NDC, 256], BF16, kind="ExternalInput").ap()
    wk_d = nc.dram_tensor("wk", [128, NDC, 256], BF16, kind="ExternalInput").ap()
    wv_d = nc.dram_tensor("wv", [128, NDC, 256], BF16, kind="ExternalInput").ap()
    wog_d = nc.dram_tensor("wog", [128, 8, D], BF16, kind="ExternalInput").ap()
    bqk_d = nc.dram_tensor("bqk", [128, 4], F32, kind="ExternalInput").ap()
    pb_d = nc.dram_tensor("pb", [128, 2], F32, kind="ExternalInput").ap()
    ones_d = nc.dram_tensor("ones", [128, 64], F32R, kind="ExternalInput").ap()
    eb_d = nc.dram_tensor("eb", [128, NB, 1024], BF16, kind="ExternalInput").ap()
    y_d = nc.dram_tensor("y", [512, D], F32, kind="ExternalOutput").ap()
    if DEBUG_DUMP:
        dbg_d = nc.dram_tensor("dbg", [128, T + 16], BF16,
                               kind="ExternalOutput").ap()
        dbgv_d = nc.dram_tensor("dbgv", [128, NKC, 4, 65], BF16,
                                kind="ExternalOutput").ap()
        dbgq_d = nc.dram_tensor("dbgq", [128, 2, T], BF16,
                                kind="ExternalOutput").ap()
        dbgk_d = nc.dram_tensor("dbgk", [128, 2, T], BF16,
                                kind="ExternalOutput").ap()

    import collections as _collections

    with tile.TileContext(nc) as tc:
        with (
            tc.tile_pool(name="const", bufs=1) as const,
            tc.tile_pool(name="persist", bufs=1) as persist,
            tc.tile_pool(name="xp", bufs=8) as xp,
            tc.tile_pool(name="pp", bufs=8) as pp,
            tc.tile_pool(name="ctxnp", bufs=4) as ctxnp,
            tc.tile_pool(name="ctxsp", bufs=4) as ctxsp,
            tc.tile_pool(name="rbp", bufs=2) as rbp,
            tc.tile_pool(name="bcp", bufs=2) as bcp,
            tc.tile_pool(name="yevac", bufs=4) as yevac,
            tc.tile_pool(name="scps", bufs=2, space="PSUM") as scps,
            tc.tile_pool(name="ctxps", bufs=2, space="PSUM") as ctxps,
        ):
            # ---- constants ----
            wq = const.tile([128, NDC, 256], BF16, tag="wq")
            wk = const.tile([128, NDC, 256], BF16, tag="wk")
            wv = const.tile([128, NDC, 256], BF16, tag="wv")
            wog = const.tile([128, 8, D], BF16, tag="wog")
            bqk = const.tile([128, 4], F32, tag="bqk")
            pbt = const.tile([128, 2], F32, tag="pb")
            ones_r = const.tile([128, 64], F32R, tag="ones_r")
            eb = const.tile([128, NB, 1024], BF16, tag="eb")
            # DMA order = need order: wk (K-pass first), then wq/wv, then
            # the attention-phase constants (first-qs boundary eb blocks
            # early, the rest + wog behind everything x-critical).
            nc.sync.dma_start(wk[:], wk_d[:])
            nc.sync.dma_start(bqk[:], bqk_d[:])
            nc.sync.dma_start(ones_r[:], ones_d[:])

            def mid_const_dmas():
                nc.sync.dma_start(wq[:], wq_d[:])
                nc.sync.dma_start(wv[:], wv_d[:])
                nc.sync.dma_start(pbt[:], pb_d[:])
                # first 6 eb entries are the qs=0 boundary blocks
                nc.sync.dma_start(eb[:, 0:6, :], eb_d[:, 0:6, :])

            def late_const_closures():
                # eb thirds + wog as closures so the deferred-slice x
                # re-DMAs interleave with them in the DMA queue instead of
                # stalling ~20us behind 7.8MB of attention constants
                cls = []
                nq = (NB - 6 + 2) // 3
                for i0 in range(6, NB, nq):
                    i1 = min(i0 + nq, NB)
                    cls.append(lambda i0=i0, i1=i1: nc.sync.dma_start(
                        eb[:, i0:i1, :], eb_d[:, i0:i1, :]))
                cls.append(lambda: nc.sync.dma_start(wog[:], wog_d[:]))
                return cls

            for _r in range(repeats):
                qT = [persist.tile([128, T], BF16, tag=f"qT{i}", name=f"qT{i}_{_r}") for i in range(2)]
                kT = [persist.tile([128, T], BF16, tag=f"kT{i}", name=f"kT{i}_{_r}") for i in range(2)]
                v_sb = persist.tile([128, NKC, 4, 65], BF16, tag="v_sb")
                nc.vector.memset(v_sb[:], 1.0)

                def xc_dmas(s):
                    xcs = []
                    for c in range(NDC):
                        xc = xp.tile([128, 512], BF16, tag="xc",
                                     name=f"xc_{_r}_{s}_{c}")
                        nc.sync.dma_start(
                            xc[:], xT[c * 128:(c + 1) * 128,
                                      s * 512:(s + 1) * 512])
                        xcs.append(xc)
                    return xcs

                ssl = lambda s: slice(s * 512, (s + 1) * 512)

                def k_alloc(s):
                    return [ctxps.tile([128, 512], F32, tag=t, bufs=1,
                                       name=f"kps{hp}_{_r}_{s}")
                            for hp, t in ((0, "bc"), (1, "y"))]

                def k_mms(s, xcs, k_ps, c0, c1):
                    for c in range(c0, c1):
                        st, sp = (c == 0), (c == NDC - 1)
                        for hp in range(2):
                            nc.tensor.matmul(
                                k_ps[hp][:],
                                wk[:, c, hp * 128:(hp + 1) * 128],
                                xcs[c][:], start=st, stop=sp)

                def k_evac(s, k_ps):
                    for hp in range(2):
                        if biases_zero:
                            nc.vector.tensor_copy(
                                kT[hp][:, ssl(s)], k_ps[hp][:])
                        else:
                            nc.scalar.activation(
                                kT[hp][:, ssl(s)], k_ps[hp][:],
                                AF.Identity, bias=bqk[:, 2 + hp:3 + hp])

                def k_slice(s, xcs):
                    k_ps = k_alloc(s)
                    k_mms(s, xcs, k_ps, 0, NDC)
                    k_evac(s, k_ps)

                def q_mms(s, xcs, q_ps, c0, c1):
                    for c in range(c0, c1):
                        st, sp = (c == 0), (c == NDC - 1)
                        for hp in range(2):
                            nc.tensor.matmul(
                                q_ps[hp][:],
                                wq[:, c, hp * 128:(hp + 1) * 128],
                                xcs[c][:], start=st, stop=sp)

                def q_evac(s, q_ps):
                    for hp in range(2):
                        if biases_zero:
                            nc.vector.tensor_copy(
                                qT[hp][:, ssl(s)], q_ps[hp][:])
                        else:
                            nc.scalar.activation(
                                qT[hp][:, ssl(s)], q_ps[hp][:],
                                AF.Identity, bias=bqk[:, hp:hp + 1])

                # v accumulators must be one-accumulation-group-per-PSUM-bank:
                # start=True clears at bank granularity, so packing two tb
                # groups into one 2KB bank clobbers the partner's partials
                def v_mms(s, xcs, v_ps, tbp, c0, c1):
                    for c in range(c0, c1):
                        st, sp = (c == 0), (c == NDC - 1)
                        for i in range(2):
                            tb = 2 * tbp + i
                            nc.tensor.matmul(
                                v_ps[i][:, 0:256],
                                xcs[c][:, tb * 128:(tb + 1) * 128],
                                wv[:, c, :], start=st, stop=sp)

                def v_copy(s, v_ps, tbp):
                    for i in range(2):
                        tb = 2 * tbp + i
                        kc = s * 4 + tb
                        vsrc = v_ps[i][:, 0:256].rearrange(
                            "p (h j) -> p h j", h=4)
                        nc.vector.tensor_copy(v_sb[:, kc, :, 0:64], vsrc[:])

                def v_closures(s, box):
                    cls = []
                    for tbp in range(2):
                        def valloc(s=s, box=box, tbp=tbp):
                            box['v'] = [ctxps.tile([128, 256], F32, tag=t,
                                                   bufs=1,
                                                   name=f"vps{i}_{_r}_{s}_{tbp}")
                                        for i, t in ((0, "bc"), (1, "y"))]
                        cls.append(valloc)
                        for c0 in range(0, NDC, 4):
                            cls.append(lambda s=s, box=box, c0=c0, tbp=tbp:
                                       v_mms(s, box['x'], box['v'], tbp,
                                             c0, c0 + 4))
                        cls.append(lambda s=s, box=box, tbp=tbp:
                                   v_copy(s, box['v'], tbp))
                    return cls

                def q_closures(s, box):
                    def qalloc(s=s, box=box):
                        box['q'] = [ctxps.tile([128, 512], F32, tag=t,
                                               bufs=1, name=f"qps{hp}_{_r}_{s}")
                                    for hp, t in ((0, "bc"), (1, "y"))]

                    cls = [qalloc]
                    for c0 in range(0, NDC, 4):
                        cls.append(lambda s=s, box=box, c0=c0:
                                   q_mms(s, box['x'], box['q'], c0, c0 + 4))
                    cls.append(lambda s=s, box=box: q_evac(s, box['q']))
                    return cls

                def kv_slice_closures(s, with_q):
                    """Deferred K+V (+Q for slice 1) for a slice: one x
                    re-DMA shared by all passes, drained during qs0."""
                    box = {}

                    def dmas(s=s, box=box):
                        box['x'] = xc_dmas(s)
                        box['k'] = k_alloc(s)

                    cls = [dmas]
                    for c0 in range(0, NDC, 4):
                        cls.append(lambda s=s, box=box, c0=c0:
                                   k_mms(s, box['x'], box['k'], c0, c0 + 4))
                    cls.append(lambda s=s, box=box: k_evac(s, box['k']))
                    cls += v_closures(s, box)
                    if with_q:
                        cls += q_closures(s, box)
                    return cls

                def q_slice_closures(s):
                    """Deferred Q-only for a slice (own x re-DMA), drained
                    one q-slice before it's consumed."""
                    box = {}
                    cls = [lambda s=s, box=box: box.__setitem__(
                        'x', xc_dmas(s))]
                    cls += q_closures(s, box)
                    return cls

                # ---- prelude: K0 + Q0 only -- everything else deferred
                xcs0 = xc_dmas(0)
                mid_const_dmas()
                k_slice(0, xcs0)
                q_ps0 = [ctxps.tile([128, 512], F32, tag=t, bufs=1,
                                    name=f"qps{hp}_{_r}_p0")
                         for hp, t in ((0, "bc"), (1, "y"))]
                q_mms(0, xcs0, q_ps0, 0, NDC)
                q_evac(0, q_ps0)
                ebcls = late_const_closures()

                # ---- fused attention + deferred QKV/norm/out-proj ----
                pending = _collections.deque()

                def drain(n):
                    k = min(n, len(pending))
                    for _ in range(k):
                        pending.popleft()()
                    return k

                dbg_keep = []
                carry = []
                for hp in range(2):
                    hlA, hlB = 2 * hp, 2 * hp + 1
                    # ctxn2[0:64, 1+q] and (duplicated) [64:128, q] per head
                    ctxn2 = [ctxnp.tile([128, T + 16], BF16, tag="ctxn",
                                        name=f"ctxn2_{_r}_{hp}_{i}")
                             for i in range(2)]
                    if hp == 0 and not dbg_keep:
                        dbg_keep.append(ctxn2[0])

                    for qs in range(4):
                        if hp == 0 and qs == 0:
                            # every qs scans all 16 k-chunks, so K and V for
                            # ALL remaining slices must land during qs0
                            # (scores chunk c needs kT[c], AV needs v_sb[c]);
                            # Q1 rides along on slice 1's x re-DMA, Q2/Q3 are
                            # deferred to later q-slices to keep qs0's PE
                            # load down; eb-table DMA thirds interleave so x
                            # transfers aren't queued behind them
                            box0 = {'x': xcs0}
                            pending.extend(v_closures(0, box0))
                            for s in range(1, 4):
                                pending.extend(kv_slice_closures(s, s == 1))
                                pending.append(ebcls[s - 1])
                            pending.append(ebcls[3])
                        if hp == 0 and qs in (1, 2):
                            pending.extend(q_slice_closures(qs + 1))
                        ctxA = ctxps.tile([65, 512], F32, tag="ctx",
                                          name=f"ctxA_{_r}_{hp}_{qs}")
                        ctxB = ctxps.tile([65, 512], F32, tag="ctx",
                                          name=f"ctxB_{_r}_{hp}_{qs}")
                        pts = {}

                        def emit_av(cc, hlA=hlA, hlB=hlB, ctxA=ctxA,
                                    ctxB=ctxB, pts=pts):
                            pm = pts.pop(cc)
                            st, sp = (cc == 0), (cc == NKC - 1)
                            nc.tensor.matmul(
                                ctxA[:], v_sb[:, cc, hlA, :],
                                pm[:, 0:512], start=st, stop=sp)
                            nc.tensor.matmul(
                                ctxB[:], v_sb[:, cc, hlB, :],
                                pm[:, 512:1024], start=st, stop=sp)

                        qsl = slice(qs * 512, (qs + 1) * 512)
                        for c in range(NKC):
                            sc = scps.tile([128, 1024], F32, tag="sc")
                            nc.tensor.matmul(
                                sc[:, 0:512],
                                kT[hp][0:64, c * 128:(c + 1) * 128],
                                qT[hp][0:64, qsl],
                                start=True, stop=True)
                            nc.tensor.matmul(
                                sc[:, 512:1024],
                                kT[hp][64:128, c * 128:(c + 1) * 128],
                                qT[hp][64:128, qsl],
                                start=True, stop=True)
                            p_t = pp.tile([128, 1024], BF16, tag="p")
                            cls = CHUNK_CLS[(c, qs)]
                            if cls == 'boundary':
                                nc.scalar.activation(p_t[:], sc[:], AF.Exp,
                                                     scale=0.125)
                                nc.vector.tensor_mul(
                                    p_t[:], p_t[:], eb[:, BIDX[(c, qs)], :])
                            else:
                                col = 0 if cls == 'low' else 1
                                nc.scalar.activation(p_t[:], sc[:], AF.Exp,
                                                     scale=0.125,
                                                     bias=pbt[:, col:col + 1])
                            pts[c] = p_t
                            if c == 0 and carry:
                                # last AV pair of the previous q-slice: it
                                # waits on that slice's final exp, so emit
                                # it AFTER this slice's first scores or the
                                # in-order PE queue would bubble ~1us at
                                # every qs boundary
                                carry.pop()()
                            if c >= 1:
                                emit_av(c - 1)
                            # qs0 must absorb the K/V bundles fast (hard
                            # dependencies at c=4s); elsewhere 1/iter keeps
                            # the PE from starving ACT in bursts.  When the
                            # queue runs dry, LDWEIGHTS-only dummies keep
                            # the PE array active for the HAM clock gate.
                            if hp == 0 and qs == 0:
                                done = drain(5 if c < 12 else 2)
                            else:
                                done = drain(1)
                            if done == 0:
                                for _ in range(3):
                                    nc.tensor.ldweights(kT[hp][:, 0:128])
                        def finish_qs(emit_av=emit_av, ctxA=ctxA, ctxB=ctxB,
                                      ctxn2=ctxn2, qs=qs, qsl=qsl, hp=hp):
                            emit_av(NKC - 1)
                            # evacuate ctx (+denominator rows) to one SBUF
                            # bf16 tile [65, 1024] (A | B); norm deferred
                            cs = ctxsp.tile([65, 1024], BF16, tag="ctxs",
                                            name=f"ctxs_{_r}_{hp}_{qs}")
                            nc.vector.tensor_copy(cs[:, 0:512], ctxA[:])
                            nc.vector.tensor_copy(cs[:, 512:1024], ctxB[:])

                            # normalization: 1/sum = exp(-ln(sum)) on ACT,
                            # batched over both heads (one FD=1024 Ln + Exp)
                            def norm_ops(cs=cs, ctxn2=ctxn2, qs=qs, qsl=qsl,
                                         key=f"{_r}_{hp}_{qs}"):
                                rbl = rbp.tile([65, 1024], F32, tag="rbl",
                                               name=f"rbl_{key}")
                                rbr = rbp.tile([65, 1024], F32R, tag="rbr",
                                               name=f"rbr_{key}")
                                nc.scalar.activation(rbl[64:65, 0:1024],
                                                     cs[64:65, :], AF.Ln)
                                nc.scalar.activation(rbr[64:65, 0:1024],
                                                     rbl[64:65, 0:1024],
                                                     AF.Exp, scale=-1.0)
                                for half in range(2):
                                    hsl = slice(half * 512, (half + 1) * 512)
                                    bc_ps = ctxps.tile([64, 512], F32,
                                                       tag="bc", bufs=1,
                                                       name=f"bcps_{key}_{half}")
                                    nc.tensor.matmul(
                                        bc_ps[:],
                                        ones_r[64:65, 0:64],
                                        rbr[64:65, hsl],
                                        start=True, stop=True)
                                    bc_sb = bcp.tile([64, 512], BF16,
                                                     tag="bc",
                                                     name=f"bcsb_{key}_{half}")
                                    nc.vector.tensor_copy(bc_sb[:, 0:512],
                                                          bc_ps[:])
                                    nc.vector.tensor_mul(
                                        ctxn2[half][0:64,
                                                    1 + qs * 512:
                                                    1 + qs * 512 + 512],
                                        cs[0:64, hsl], bc_sb[:, 0:512])
                                    # duplicate (unshifted) into rows 64:128
                                    nc.sync.dma_start(
                                        ctxn2[half][64:128, qsl],
                                        ctxn2[half][0:64,
                                                    1 + qs * 512:
                                                    1 + qs * 512 + 512])
                            pending.append(norm_ops)
                        carry.append(finish_qs)

                    # flush the last q-slice's finisher before the out-proj
                    # closures are queued (they read all of ctxn2)
                    while carry:
                        carry.pop()()

                    # K=128 out-projections: 8 u-chunks x 2 ds per head
                    for hoff in range(2):
                        hl = 2 * hp + hoff
                        r2 = ctxn2[hoff].rearrange("p (tc s) -> p s tc", s=16)
                        for ds in range(2):
                            ypsb = []

                            def yalloc(hl=hl, ds=ds, ypsb=ypsb):
                                ypsb.append(ctxps.tile(
                                    [128, 512], F32, tag="y", bufs=1,
                                    name=f"yps_{_r}_{hl}_{ds}"))

                            if OUTPROJ_K64:
                                def ymm(u0, r2=r2, ds=ds, ypsb=ypsb):
                                    for tf in range(2 * u0, 2 * u0 + 8):
                                        par = tf % 2
                                        nc.tensor.matmul(
                                            ypsb[0][:],
                                            r2[64 * par:64 * par + 64,
                                               tf + 1 - par, 0:128],
                                            wog[64 * par:64 * par + 64,
                                                tf // 2,
                                                ds * 512:(ds + 1) * 512],
                                            start=(tf == 0), stop=(tf == 15))
                            else:
                                def ymm(u0, r2=r2, ds=ds, ypsb=ypsb):
                                    for u in range(u0, u0 + 4):
                                        nc.tensor.matmul(
                                            ypsb[0][:],
                                            r2[:, 2 * u + 1, 0:128],
                                            wog[:, u, ds * 512:(ds + 1) * 512],
                                            start=(u == 0), stop=(u == 7))

                            def yout(hl=hl, ds=ds, ypsb=ypsb):
                                ysb = yevac.tile([128, 512], F32, tag="y",
                                                 name=f"ysb_{_r}_{hl}_{ds}")
                                nc.vector.tensor_copy(ysb[:], ypsb[0][:])
                                nc.sync.dma_start(
                                    y_d[hl * 128:(hl + 1) * 128,
                                        ds * 512:(ds + 1) * 512],
                                    ysb[:])

                            pending.append(yalloc)
                            for u0 in (0, 4):
                                pending.append(
                                    lambda u0=u0, ymm=ymm: ymm(u0))
                            pending.append(yout)

                # tail: whatever the last head pair's attention didn't absorb
                drain(len(pending))
                if DEBUG_DUMP:
                    nc.sync.dma_start(dbg_d[:], dbg_keep[0][:])
                    nc.sync.dma_start(dbgv_d[:], v_sb[:])
                    for i in range(2):
                        nc.sync.dma_start(dbgq_d[:, i, :], qT[i][:])
                        nc.sync.dma_start(dbgk_d[:, i, :], kT[i][:])
    nc.compile()
    return nc


_PROGRAM_CACHE = {}


def _get_program(repeats=1, biases_zero=True):
    key = (repeats, biases_zero)
    if key not in _PROGRAM_CACHE:
        _PROGRAM_CACHE[key] = build_program(repeats=repeats,
                                            biases_zero=biases_zero)
    return _PROGRAM_CACHE[key]


def kernel(**inputs):
    from concourse.bass_utils import run_bass_kernel_spmd
    in_maps, post, biases_zero = host_prep(inputs)
    nc = _get_program(repeats=1, biases_zero=biases_zero)
    res = run_bass_kernel_spmd(nc, in_maps, list(range(8)))
    return post(res.results)


# revision 49
# speedup vs baseline: 1.0897x; 1.0642x over previous
"""BTSPAttention Trainium2 kernel for 8 NeuronCores (self-contained).

Usage: kernel(**inputs) -> np.ndarray  (full [2,2048,1024] float32 output)

Sharding: 8 cores = 2 batches x 4 head-groups (4 heads each).

v4 architecture -- single fused emission stream, ACT(exp)-limited:
  All matmul operands are bf16 (fp32r streams 2 PE passes on HW; bf16 is 1
  pass + FWL weight loads).  The scalar engine is the critical resource
  (~128 exps of [128,1024] at ~1.1us each), so everything else is arranged
  to keep its FIFO fed with exps back-to-back:
  - K-projection runs first (kT needed by every score chunk), then Q/V for
    slice 0, then attention begins; the remaining Q/V slices re-DMA their
    x chunks and run as deferred closures drained between attention
    iterations (engine FIFOs execute in emission order).
  - Scores per (head-pair, qs, k-chunk c): one [128,1024] PSUM tile, two
    K=64 matmuls in disjoint PE row groups (concurrent), one FD=1024 exp
    for both heads; pure-clip chunks fold the time-bias into the exp bias,
    boundary chunks multiply by a precomputed bf16 Toeplitz block on DVE.
  - ctx accum [65,512] = [V_h|1]^T @ P (row 64 = softmax denominator);
    evacuated to SBUF bf16 immediately; normalization (exp(-ln(sum)) on
    ACT, PE ones-broadcast, DVE multiply) is deferred onto the queue.
  - Out-projection at K=128: ctxn2 [128, T+16] holds the head's context
    twice -- rows 0:64 at column q+1, rows 64:128 (DMA-duplicated) at
    column q -- so the AP [128 part, col 2u+1 step 16] is exactly the
    (tf=2u | tf=2u+1, j) contraction block; 8 K=128 matmuls per (head, ds)
    replace 16 K=64 ones.  wog3[64*(tf%2)+j, tf//2, do] = Wo.T grouping.
  - Zero-bias fast path (host-detected): Q/K evacuations are DVE copies;
    the general path uses ACT Identity+bias.  bk and is_gate are dropped
    exactly (softmax shift invariance); bv and bo applied on the host.
PSUM tags: "sc" 2x[128,1024] (scores), "ctx" 2x[65,512] (ctx accum),
"bc" + "y" 1x[128,512]-class each (K/Q/V deferred accums, norm
broadcast, out-proj accum).  8 banks exactly.  NOTE: matmul start=True
clears PSUM at BANK granularity -- never pack two accumulation groups
into one 2KB bank (the V-pass runs as two tb-pair sub-passes for this).
"""

import numpy as np
import ml_dtypes

import sys as _sys
if '/opt/trn_rl_repo' not in _sys.path:
    _sys.path.insert(0, '/opt/trn_rl_repo')


import concourse.bass as bass
import concourse.tile as tile
from concourse import bacc
from concourse import mybir

F32 = mybir.dt.float32
F32R = mybir.dt.float32r
BF16 = mybir.dt.bfloat16
AF = mybir.ActivationFunctionType

T = 2048
D = 1024
HD = 64
TB_LEN = 500
NKC = 16   # k chunks of 128
NDC = 8    # D chunks of 128

# ---- structural chunk classification (depends only on the clip pattern) ----
# scoresT chunk (c, qs): k in [128c, 128c+128), q in [512qs, 512qs+512).
# E[k, q] = exp(sig * tb[clip(k - q + 250, 0, 499)]).
# pure-low  (idx pinned 0):   k - q <= -250 everywhere  -> E = exp(sig*tb[0])
# pure-high (idx pinned 499): k - q >= 249 everywhere   -> E = exp(sig*tb[499])
def _classify(c, qs):
    kmin, kmax = 128 * c, 128 * c + 127
    qmin, qmax = 512 * qs, 512 * qs + 511
    if kmax - qmin <= -250:
        return 'low'
    if kmin - qmax >= 249:
        return 'high'
    return 'boundary'

CHUNK_CLS = {(c, qs): _classify(c, qs) for c in range(NKC) for qs in range(4)}
BOUNDARY = [(c, qs) for qs in range(4) for c in range(NKC)
            if CHUNK_CLS[(c, qs)] == 'boundary']
BIDX = {cq: i for i, cq in enumerate(BOUNDARY)}
NB = len(BOUNDARY)  # 28
OUTPROJ_K64 = False  # debug bisect: revert to v3-style K=64 out-projection
DEBUG_DUMP = False   # dump head-0 ctxn2 layout to 'dbg' output


def host_prep(inputs):
    """Returns (in_maps for 8 cores, postprocess-closure, biases_zero)."""
    x = np.asarray(inputs["x"], np.float32)
    Wq = np.asarray(inputs["Wq"], np.float32)
    Wk = np.asarray(inputs["Wk"], np.float32)
    Wv = np.asarray(inputs["Wv"], np.float32)
    Wo = np.asarray(inputs["Wo"], np.float32)
    bq = np.asarray(inputs["bq"], np.float32)
    bk = np.asarray(inputs["bk"], np.float32)
    bv = np.asarray(inputs["bv"], np.float32)
    bo = np.asarray(inputs["bo"], np.float32)
    et = float(np.asarray(inputs["et_gate"], np.float32).reshape(()))
    tb = np.asarray(inputs["time_bias"], np.float32).reshape(-1)
    assert tb.shape == (TB_LEN,)
    # bk shifts every score of a query by the same amount -> softmax
    # invariant -> dropped exactly.  bq is not invariant; when nonzero the
    # program variant with ACT-bias evacuation is used.
    biases_zero = not (np.any(bq) or np.any(bk))

    sig = 1.0 / (1.0 + np.exp(-et))
    idx = np.clip(np.arange(T)[:, None] - np.arange(T)[None, :] + TB_LEN // 2,
                  0, TB_LEN - 1)              # [k, q]
    E = np.exp(np.float32(sig) * tb[idx]).astype(np.float32)
    # boundary-chunk Toeplitz table, duplicated halves (head A | head B
    # of a pair share the same (c, qs) block): [128, NB, 1024]
    ebb = np.empty((128, NB, 1024), np.float32)
    for i, (c, qs) in enumerate(BOUNDARY):
        blk = E[128 * c:128 * c + 128, 512 * qs:512 * qs + 512]
        ebb[:, i, 0:512] = blk
        ebb[:, i, 512:1024] = blk
    ebb = ebb.astype(ml_dtypes.bfloat16)

    # exp bias for pure chunks: log E = sig * tb[0 or 499]
    pb = np.zeros((128, 2), np.float32)
    pb[:, 0] = sig * tb[0]           # pure-low
    pb[:, 1] = sig * tb[TB_LEN - 1]  # pure-high

    # wog3[64*(tf%2)+j, tf//2, do] = Wo.T[64tf+j, do]: the (tf-parity, j)
    # contraction grouping for the K=128 out-projection
    wg = np.ascontiguousarray(Wo.T.reshape(16, 64, D).transpose(1, 0, 2))  # [j, tf, do]
    wog3 = np.zeros((128, 8, D), np.float32)
    for tf in range(16):
        wog3[64 * (tf % 2):64 * (tf % 2) + 64, tf // 2, :] = wg[:, tf, :]
    wog3 = wog3.astype(ml_dtypes.bfloat16)

    def chunk_w(Wl):  # Wl [256, 1024] -> [128, 8, 256]: [p, c, m] = Wl[m, c*128+p]
        return np.ascontiguousarray(
            Wl.T.reshape(NDC, 128, 256).transpose(1, 0, 2)
        ).astype(ml_dtypes.bfloat16)

    in_maps = []
    for core in range(8):
        b, hg = core // 4, core % 4
        sl = slice(hg * 256, (hg + 1) * 256)
        bqk = np.stack([bq[sl][:128], bq[sl][128:],
                        bk[sl][:128], bk[sl][128:]], axis=1)  # [128, 4]
        in_maps.append({
            "xT": np.ascontiguousarray(x[b].T).astype(ml_dtypes.bfloat16),
            "wq": chunk_w(Wq[sl]),
            "wk": chunk_w(Wk[sl]),
            "wv": chunk_w(Wv[sl]),
            "wog": wog3,
            "bqk": np.ascontiguousarray(bqk, np.float32),
            "pb": pb,
            "ones": np.ones((128, 64), np.float32),
            "eb": ebb,
        })

    corr = np.einsum("hj,jfd->hd", bv.reshape(16, HD), wg).astype(np.float32)  # per global head

    def post(results):
        out = np.empty((2, T, D), np.float32)
        for core in range(8):
            b, hg = core // 4, core % 4
            yc = results[core]["y"]  # [512, 1024]
            for hl in range(4):
                h = hg * 4 + hl
                rows = (h % 8) * 256 + b * 128
                out[h // 8, rows:rows + 128, :] = (
                    yc[hl * 128:(hl + 1) * 128] + corr[h][None, :] + bo[None, :]
                )
        return out

    return in_maps, post, biases_zero


def expected_core(inputs, core):
    """Numpy model of one core's device output (for sim checks)."""
    m, _, _ = host_prep(inputs)
    im = m[core]
    et = float(np.asarray(inputs["et_gate"], np.float32).reshape(()))
    tb = np.asarray(inputs["time_bias"], np.float32).reshape(-1)
    sig = 1.0 / (1.0 + np.exp(-et))
    idx = np.clip(np.arange(T)[:, None] - np.arange(T)[None, :] + TB_LEN // 2,
                  0, TB_LEN - 1)
    E = np.exp(np.float32(sig) * tb[idx]).astype(np.float32)
    y = np.zeros((512, 1024), np.float32)
    bqk = im["bqk"]
    xT = np.asarray(im["xT"], np.float32)

    def _bf(a):
        return a.astype(ml_dtypes.bfloat16).astype(np.float32)

    wq = np.asarray(im["wq"], np.float32)
    wk = np.asarray(im["wk"], np.float32)
    wv = np.asarray(im["wv"], np.float32)
    Wq_l = np.concatenate([wq[:, c, :] for c in range(NDC)], axis=0)  # [1024, 256] = Wl.T
    Wk_l = np.concatenate([wk[:, c, :] for c in range(NDC)], axis=0)
    Wv_l = np.concatenate([wv[:, c, :] for c in range(NDC)], axis=0)
    QT = _bf(Wq_l.T @ xT + np.concatenate([bqk[:, 0], bqk[:, 1]])[:, None])
    KT = _bf(Wk_l.T @ xT + np.concatenate([bqk[:, 2], bqk[:, 3]])[:, None])
    V = xT.T @ Wv_l
    wog3 = np.asarray(im["wog"], np.float32)  # [128, 8, 1024]
    wg = np.zeros((64, 16, D), np.float32)
    for tf in range(16):
        wg[:, tf, :] = wog3[64 * (tf % 2):64 * (tf % 2) + 64, tf // 2, :]
    for hl in range(4):
        qh_ = QT[hl * 64:(hl + 1) * 64]
        kh = KT[hl * 64:(hl + 1) * 64]
        P = np.exp(0.125 * (kh.T @ qh_)) * E
        c = (V[:, hl * 64:(hl + 1) * 64].T @ P) / P.sum(axis=0)[None, :]  # [64, q]
        cn = _bf(c)
        g = cn.reshape(64, 128, 16)
        y[hl * 128:(hl + 1) * 128] = np.einsum("jcf,jfd->cd", g, wg)
    return y


def build_program(repeats=1, biases_zero=True):
    nc = bacc.Bacc("TRN2", target_bir_lowering=False, debug=False,
                   dynamic_dma_scratch_size=4096)

    # All activation functions used here (Exp, Ln, Copy, Identity) live in
    # the natural_log_exp_and_others table set, but walrus's first-match set
    # selection would ping-pong between exp_and_others and natural_log
    # (one ~1.3us ACT_TABLE_LOAD per Ln/Exp alternation, 17 loads/kernel).
    # Restrict the offered tables so a single load covers the whole kernel.
    import types as _types

    def _single_act_set(self):
        has_activation = any(
            isinstance(i, mybir.InstActivation)
            for b in self.main_func.blocks
            for i in b.instructions
        )
        if not has_activation:
            return
        from concourse.hw_specs import get_activation_tables
        tables = [(n, f if n == 'natural_log_exp_and_others' else set())
                  for n, f in get_activation_tables(self.m.arch).items()]
        assert any(f for _, f in tables), "natural_log_exp_and_others missing"
        bacc._bass_rust.insert_act_table_loads(self, tables)

    nc.insert_act_table_loads = _types.MethodType(_single_act_set, nc)
    xT = nc.dram_tensor("xT", [D, T], BF16, kind="ExternalInput").ap()
    wq_d = nc.dram_tensor("wq", [128, NDC, 256], BF16, kind="ExternalInput").ap()
    wk_d = nc.dram_tensor("wk", [128, NDC, 256], BF16, kind="ExternalInput").ap()
    wv_d = nc.dram_tensor("wv", [128, NDC, 256], BF16, kind="ExternalInput").ap()
    wog_d = nc.dram_tensor("wog", [128, 8, D], BF16, kind="ExternalInput").ap()
    bqk_d = nc.dram_tensor("bqk", [128, 4], F32, kind="ExternalInput").ap()
    pb_d = nc.dram_tensor("pb", [128, 2], F32, kind="ExternalInput").ap()
    ones_d = nc.dram_tensor("ones", [128, 64], F32R, kind="ExternalInput").ap()
    eb_d = nc.dram_tensor("eb", [128, NB, 1024], BF16, kind="ExternalInput").ap()
    y_d = nc.dram_tensor("y", [512, D], F32, kind="ExternalOutput").ap()
    if DEBUG_DUMP:
        dbg_d = nc.dram_tensor("dbg", [128, T + 16], BF16,
                               kind="ExternalOutput").ap()
        dbgv_d = nc.dram_tensor("dbgv", [128, NKC, 4, 65], BF16,
                                kind="ExternalOutput").ap()
        dbgq_d = nc.dram_tensor("dbgq", [128, 2, T], BF16,
                                kind="ExternalOutput").ap()
        dbgk_d = nc.dram_tensor("dbgk", [128, 2, T], BF16,
                                kind="ExternalOutput").ap()

    import collections as _collections

    with tile.TileContext(nc) as tc:
        with (
            tc.tile_pool(name="const", bufs=1) as const,
            tc.tile_pool(name="persist", bufs=1) as persist,
            tc.tile_pool(name="xp", bufs=8) as xp,
            tc.tile_pool(name="pp", bufs=8) as pp,
            tc.tile_pool(name="ctxnp", bufs=4) as ctxnp,
            tc.tile_pool(name="ctxsp", bufs=4) as ctxsp,
            tc.tile_pool(name="rbp", bufs=2) as rbp,
            tc.tile_pool(name="bcp", bufs=2) as bcp,
            tc.tile_pool(name="yevac", bufs=4) as yevac,
            tc.tile_pool(name="scps", bufs=2, space="PSUM") as scps,
            tc.tile_pool(name="ctxps", bufs=2, space="PSUM") as ctxps,
        ):
            # ---- constants ----
            wq = const.tile([128, NDC, 256], BF16, tag="wq")
            wk = const.tile([128, NDC, 256], BF16, tag="wk")
            wv = const.tile([128, NDC, 256], BF16, tag="wv")
            wog = const.tile([128, 8, D], BF16, tag="wog")
            bqk = const.tile([128, 4], F32, tag="bqk")
            pbt = const.tile([128, 2], F32, tag="pb")
            ones_r = const.tile([128, 64], F32R, tag="ones_r")
            eb = const.tile([128, NB, 1024], BF16, tag="eb")
            # DMA order = need order: wk (K-pass first), then wq/wv, then
            # the attention-phase constants (first-qs boundary eb blocks
            # early, the rest + wog behind everything x-critical).
            nc.sync.dma_start(wk[:], wk_d[:])
            nc.sync.dma_start(bqk[:], bqk_d[:])
            nc.sync.dma_start(ones_r[:], ones_d[:])

            def mid_const_dmas():
                nc.sync.dma_start(wq[:], wq_d[:])
                nc.sync.dma_start(wv[:], wv_d[:])
                nc.sync.dma_start(pbt[:], pb_d[:])
                # first 6 eb entries are the qs=0 boundary blocks
                nc.sync.dma_start(eb[:, 0:6, :], eb_d[:, 0:6, :])

            def late_const_closures():
                # eb thirds + wog as closures so the deferred-slice x
                # re-DMAs interleave with them in the DMA queue instead of
                # stalling ~20us behind 7.8MB of attention constants
                cls = []
                nq = (NB - 6 + 2) // 3
                for i0 in range(6, NB, nq):
                    i1 = min(i0 + nq, NB)
                    cls.append(lambda i0=i0, i1=i1: nc.sync.dma_start(
                        eb[:, i0:i1, :], eb_d[:, i0:i1, :]))
                cls.append(lambda: nc.sync.dma_start(wog[:], wog_d[:]))
                return cls

            for _r in range(repeats):
                qT = [persist.tile([128, T], BF16, tag=f"qT{i}", name=f"qT{i}_{_r}") for i in range(2)]
                kT = [persist.tile([128, T], BF16, tag=f"kT{i}", name=f"kT{i}_{_r}") for i in range(2)]
                v_sb = persist.tile([128, NKC, 4, 65], BF16, tag="v_sb")
                nc.vector.memset(v_sb[:], 1.0)

                def xc_dmas(s):
                    xcs = []
                    for c in range(NDC):
                        xc = xp.tile([128, 512], BF16, tag="xc",
                                     name=f"xc_{_r}_{s}_{c}")
                        nc.sync.dma_start(
                            xc[:], xT[c * 128:(c + 1) * 128,
                                      s * 512:(s + 1) * 512])
                        xcs.append(xc)
                    return xcs

                ssl = lambda s: slice(s * 512, (s + 1) * 512)

                def k_alloc(s):
                    return [ctxps.tile([128, 512], F32, tag=t, bufs=1,
                                       name=f"kps{hp}_{_r}_{s}")
                            for hp, t in ((0, "bc"), (1, "y"))]

                def k_mms(s, xcs, k_ps, c0, c1):
                    for c in range(c0, c1):
                        st, sp = (c == 0), (c == NDC - 1)
                        for hp in range(2):
                            nc.tensor.matmul(
                                k_ps[hp][:],
                                wk[:, c, hp * 128:(hp + 1) * 128],
                                xcs[c][:], start=st, stop=sp)

                def k_evac(s, k_ps):
                    for hp in range(2):
                        if biases_zero:
                            nc.vector.tensor_copy(
                                kT[hp][:, ssl(s)], k_ps[hp][:])
                        else:
                            nc.scalar.activation(
                                kT[hp][:, ssl(s)], k_ps[hp][:],
                                AF.Identity, bias=bqk[:, 2 + hp:3 + hp])

                def k_slice(s, xcs):
                    k_ps = k_alloc(s)
                    k_mms(s, xcs, k_ps, 0, NDC)
                    k_evac(s, k_ps)

                def q_mms(s, xcs, q_ps, c0, c1):
                    for c in range(c0, c1):
                        st, sp = (c == 0), (c == NDC - 1)
                        for hp in range(2):
                            nc.tensor.matmul(
                                q_ps[hp][:],
                                wq[:, c, hp * 128:(hp + 1) * 128],
                                xcs[c][:], start=st, stop=sp)

                def q_evac(s, q_ps):
                    for hp in range(2):
                        if biases_zero:
                            nc.vector.tensor_copy(
                                qT[hp][:, ssl(s)], q_ps[hp][:])
                        else:
                            nc.scalar.activation(
                                qT[hp][:, ssl(s)], q_ps[hp][:],
                                AF.Identity, bias=bqk[:, hp:hp + 1])

                # v accumulators must be one-accumulation-group-per-PSUM-bank:
                # start=True clears at bank granularity, so packing two tb
                # groups into one 2KB bank clobbers the partner's partials
                def v_mms(s, xcs, v_ps, tbp, c0, c1):
                    for c in range(c0, c1):
                        st, sp = (c == 0), (c == NDC - 1)
                        for i in range(2):
                            tb = 2 * tbp + i
                            nc.tensor.matmul(
                                v_ps[i][:, 0:256],
                                xcs[c][:, tb * 128:(tb + 1) * 128],
                                wv[:, c, :], start=st, stop=sp)

                def v_copy(s, v_ps, tbp):
                    for i in range(2):
                        tb = 2 * tbp + i
                        kc = s * 4 + tb
                        vsrc = v_ps[i][:, 0:256].rearrange(
                            "p (h j) -> p h j", h=4)
                        nc.vector.tensor_copy(v_sb[:, kc, :, 0:64], vsrc[:])

                def v_closures(s, box):
                    cls = []
                    for tbp in range(2):
                        def valloc(s=s, box=box, tbp=tbp):
                            box['v'] = [ctxps.tile([128, 256], F32, tag=t,
                                                   bufs=1,
                                                   name=f"vps{i}_{_r}_{s}_{tbp}")
                                        for i, t in ((0, "bc"), (1, "y"))]
                        cls.append(valloc)
                        for c0 in range(0, NDC, 4):
                            cls.append(lambda s=s, box=box, c0=c0, tbp=tbp:
                                       v_mms(s, box['x'], box['v'], tbp,
                                             c0, c0 + 4))
                        cls.append(lambda s=s, box=box, tbp=tbp:
                                   v_copy(s, box['v'], tbp))
                    return cls

                def q_closures(s, box):
                    def qalloc(s=s, box=box):
                        box['q'] = [ctxps.tile([128, 512], F32, tag=t,
                                               bufs=1, name=f"qps{hp}_{_r}_{s}")
                                    for hp, t in ((0, "bc"), (1, "y"))]

                    cls = [qalloc]
                    for c0 in range(0, NDC, 4):
                        cls.append(lambda s=s, box=box, c0=c0:
                                   q_mms(s, box['x'], box['q'], c0, c0 + 4))
                    cls.append(lambda s=s, box=box: q_evac(s, box['q']))
                    return cls

                def kv_slice_closures(s, with_q):
                    """Deferred K+V (+Q for slice 1) for a slice: one x
                    re-DMA shared by all passes, drained during qs0."""
                    box = {}

                    def dmas(s=s, box=box):
                        box['x'] = xc_dmas(s)
                        box['k'] = k_alloc(s)

                    cls = [dmas]
                    for c0 in range(0, NDC, 4):
                        cls.append(lambda s=s, box=box, c0=c0:
                                   k_mms(s, box['x'], box['k'], c0, c0 + 4))
                    cls.append(lambda s=s, box=box: k_evac(s, box['k']))
                    cls += v_closures(s, box)
                    if with_q:
                        cls += q_closures(s, box)
                    return cls

                def q_slice_closures(s):
                    """Deferred Q-only for a slice (own x re-DMA), drained
                    one q-slice before it's consumed."""
                    box = {}
                    cls = [lambda s=s, box=box: box.__setitem__(
                        'x', xc_dmas(s))]
                    cls += q_closures(s, box)
                    return cls

                # ---- prelude: K0 + Q0 only -- everything else deferred
                xcs0 = xc_dmas(0)
                mid_const_dmas()
                k_slice(0, xcs0)
                q_ps0 = [ctxps.tile([128, 512], F32, tag=t, bufs=1,
                                    name=f"qps{hp}_{_r}_p0")
                         for hp, t in ((0, "bc"), (1, "y"))]
                q_mms(0, xcs0, q_ps0, 0, NDC)
                q_evac(0, q_ps0)
                ebcls = late_const_closures()

                # ---- fused attention + deferred QKV/norm/out-proj ----
                pending = _collections.deque()

                def drain(n):
                    k = min(n, len(pending))
                    for _ in range(k):
                        pending.popleft()()
                    return k

                dbg_keep = []
                carry = []
                for hp in range(2):
                    hlA, hlB = 2 * hp, 2 * hp + 1
                    # ctxn2[0:64, 1+q] and (duplicated) [64:128, q] per head
                    ctxn2 = [ctxnp.tile([128, T + 16], BF16, tag="ctxn",
                                        name=f"ctxn2_{_r}_{hp}_{i}")
                             for i in range(2)]
                    if hp == 0 and not dbg_keep:
                        dbg_keep.append(ctxn2[0])

                    for qs in range(4):
                        if hp == 0 and qs == 0:
                            # every qs scans all 16 k-chunks, so K and V for
                            # ALL remaining slices must land during qs0
                            # (scores chunk c needs kT[c], AV needs v_sb[c]);
                            # Q1 rides along on slice 1's x re-DMA, Q2/Q3 are
                            # deferred to later q-slices to keep qs0's PE
                            # load down; eb-table DMA thirds interleave so x
                            # transfers aren't queued behind them
                            box0 = {'x': xcs0}
                            pending.extend(v_closures(0, box0))
                            for s in range(1, 4):
                                pending.extend(kv_slice_closures(s, s == 1))
                                pending.append(ebcls[s - 1])
                            pending.append(ebcls[3])
                        if hp == 0 and qs in (1, 2):
                            pending.extend(q_slice_closures(qs + 1))
                        ctxA = ctxps.tile([65, 512], F32, tag="ctx",
                                          name=f"ctxA_{_r}_{hp}_{qs}")
                        ctxB = ctxps.tile([65, 512], F32, tag="ctx",
                                          name=f"ctxB_{_r}_{hp}_{qs}")
                        pts = {}

                        def emit_av(cc, hlA=hlA, hlB=hlB, ctxA=ctxA,
                                    ctxB=ctxB, pts=pts):
                            pm = pts.pop(cc)
                            st, sp = (cc == 0), (cc == NKC - 1)
                            nc.tensor.matmul(
                                ctxA[:], v_sb[:, cc, hlA, :],
                                pm[:, 0:512], start=st, stop=sp)
                            nc.tensor.matmul(
                                ctxB[:], v_sb[:, cc, hlB, :],
                                pm[:, 512:1024], start=st, stop=sp)

                        qsl = slice(qs * 512, (qs + 1) * 512)
                        for c in range(NKC):
                            sc = scps.tile([128, 1024], F32, tag="sc")
                            nc.tensor.matmul(
                                sc[:, 0:512],
                                kT[hp][0:64, c * 128:(c + 1) * 128],
                                qT[hp][0:64, qsl],
                                start=True, stop=True)
                            nc.tensor.matmul(
                                sc[:, 512:1024],
                                kT[hp][64:128, c * 128:(c + 1) * 128],
                                qT[hp][64:128, qsl],
                                start=True, stop=True)
                            p_t = pp.tile([128, 1024], BF16, tag="p")
                            cls = CHUNK_CLS[(c, qs)]
                            if cls == 'boundary':
                                nc.scalar.activation(p_t[:], sc[:], AF.Exp,
                                                     scale=0.125)
                                nc.vector.tensor_mul(
                                    p_t[:], p_t[:], eb[:, BIDX[(c, qs)], :])
                            else:
                                col = 0 if cls == 'low' else 1
                                nc.scalar.activation(p_t[:], sc[:], AF.Exp,
                                                     scale=0.125,
                                                     bias=pbt[:, col:col + 1])
                            pts[c] = p_t
                            if c == 0 and carry:
                                # last AV pair of the previous q-slice: it
                                # waits on that slice's final exp, so emit
                                # it AFTER this slice's first scores or the
                                # in-order PE queue would bubble ~1us at
                                # every qs boundary
                                carry.pop()()
                            if c >= 1:
                                emit_av(c - 1)
                            # qs0 must absorb the K/V bundles fast (hard
                            # dependencies at c=4s); elsewhere 1/iter keeps
                            # the PE from starving ACT in bursts.  When the
                            # queue runs dry, LDWEIGHTS-only dummies keep
                            # the PE array active for the HAM clock gate.
                            if hp == 0 and qs == 0:
                                done = drain(5 if c < 12 else 2)
                            else:
                                done = drain(1)
                            del done
                        def finish_qs(emit_av=emit_av, ctxA=ctxA, ctxB=ctxB,
                                      ctxn2=ctxn2, qs=qs, qsl=qsl, hp=hp):
                            emit_av(NKC - 1)
                            # evacuate ctx (+denominator rows) to one SBUF
                            # bf16 tile [65, 1024] (A | B); norm deferred
                            cs = ctxsp.tile([65, 1024], BF16, tag="ctxs",
                                            name=f"ctxs_{_r}_{hp}_{qs}")
                            nc.vector.tensor_copy(cs[:, 0:512], ctxA[:])
                            nc.vector.tensor_copy(cs[:, 512:1024], ctxB[:])

                            # normalization: 1/sum = exp(-ln(sum)) on ACT,
                            # batched over both heads (one FD=1024 Ln + Exp)
                            def norm_ops(cs=cs, ctxn2=ctxn2, qs=qs, qsl=qsl,
                                         key=f"{_r}_{hp}_{qs}"):
                                rbl = rbp.tile([65, 1024], F32, tag="rbl",
                                               name=f"rbl_{key}")
                                rbr = rbp.tile([65, 1024], F32R, tag="rbr",
                                               name=f"rbr_{key}")
                                nc.scalar.activation(rbl[64:65, 0:1024],
                                                     cs[64:65, :], AF.Ln)
                                nc.scalar.activation(rbr[64:65, 0:1024],
                                                     rbl[64:65, 0:1024],
                                                     AF.Exp, scale=-1.0)
                                for half in range(2):
                                    hsl = slice(half * 512, (half + 1) * 512)
                                    bc_ps = ctxps.tile([64, 512], F32,
                                                       tag="bc", bufs=1,
                                                       name=f"bcps_{key}_{half}")
                                    nc.tensor.matmul(
                                        bc_ps[:],
                                        ones_r[64:65, 0:64],
                                        rbr[64:65, hsl],
                                        start=True, stop=True)
                                    bc_sb = bcp.tile([64, 512], BF16,
                                                     tag="bc",
                                                     name=f"bcsb_{key}_{half}")
                                    nc.vector.tensor_copy(bc_sb[:, 0:512],
                                                          bc_ps[:])
                                    nc.vector.tensor_mul(
                                        ctxn2[half][0:64,
                                                    1 + qs * 512:
                                                    1 + qs * 512 + 512],
                                        cs[0:64, hsl], bc_sb[:, 0:512])
                                    # duplicate (unshifted) into rows 64:128
                                    nc.sync.dma_start(
                                        ctxn2[half][64:128, qsl],
                                        ctxn2[half][0:64,
                                                    1 + qs * 512:
                                                    1 + qs * 512 + 512])
                            pending.append(norm_ops)
                        carry.append(finish_qs)

                    # flush the last q-slice's finisher before the out-proj
                    # closures are queued (they read all of ctxn2)
                    while carry:
                        carry.pop()()

                    # K=128 out-projections: 8 u-chunks x 2 ds per head
                    for hoff in range(2):
                        hl = 2 * hp + hoff
                        r2 = ctxn2[hoff].rearrange("p (tc s) -> p s tc", s=16)
                        for ds in range(2):
                            ypsb = []

                            def yalloc(hl=hl, ds=ds, ypsb=ypsb):
                                ypsb.append(ctxps.tile(
                                    [128, 512], F32, tag="y", bufs=1,
                                    name=f"yps_{_r}_{hl}_{ds}"))

                            if OUTPROJ_K64:
                                def ymm(u0, r2=r2, ds=ds, ypsb=ypsb):
                                    for tf in range(2 * u0, 2 * u0 + 8):
                                        par = tf % 2
                                        nc.tensor.matmul(
                                            ypsb[0][:],
                                            r2[64 * par:64 * par + 64,
                                               tf + 1 - par, 0:128],
                                            wog[64 * par:64 * par + 64,
                                                tf // 2,
                                                ds * 512:(ds + 1) * 512],
                                            start=(tf == 0), stop=(tf == 15))
                            else:
                                def ymm(u0, r2=r2, ds=ds, ypsb=ypsb):
                                    for u in range(u0, u0 + 4):
                                        nc.tensor.matmul(
                                            ypsb[0][:],
                                            r2[:, 2 * u + 1, 0:128],
                                            wog[:, u, ds * 512:(ds + 1) * 512],
                                            start=(u == 0), stop=(u == 7))

                            def yout(hl=hl, ds=ds, ypsb=ypsb):
                                ysb = yevac.tile([128, 512], F32, tag="y",
                                                 name=f"ysb_{_r}_{hl}_{ds}")
                                nc.vector.tensor_copy(ysb[:], ypsb[0][:])
                                nc.sync.dma_start(
                                    y_d[hl * 128:(hl + 1) * 128,
                                        ds * 512:(ds + 1) * 512],
                                    ysb[:])

                            pending.append(yalloc)
                            for u0 in (0, 4):
                                pending.append(
                                    lambda u0=u0, ymm=ymm: ymm(u0))
                            pending.append(yout)

                # tail: whatever the last head pair's attention didn't absorb
                drain(len(pending))
                if DEBUG_DUMP:
                    nc.sync.dma_start(dbg_d[:], dbg_keep[0][:])
                    nc.sync.dma_start(dbgv_d[:], v_sb[:])
                    for i in range(2):
                        nc.sync.dma_start(dbgq_d[:, i, :], qT[i][:])
                        nc.sync.dma_start(dbgk_d[:, i, :], kT[i][:])
    nc.compile()
    return nc


_PROGRAM_CACHE = {}


def _get_program(repeats=1, biases_zero=True):
    key = (repeats, biases_zero)
    if key not in _PROGRAM_CACHE:
        _PROGRAM_CACHE[key] = build_program(repeats=repeats,
                                            biases_zero=biases_zero)
    return _PROGRAM_CACHE[key]


def kernel(**inputs):
    from concourse.bass_utils import run_bass_kernel_spmd
    in_maps, post, biases_zero = host_prep(inputs)
    nc = _get_program(repeats=1, biases_zero=biases_zero)
    res = run_bass_kernel_spmd(nc, in_maps, list(range(8)))
    return post(res.results)
